# revision 16
# baseline (speedup 1.0000x reference)
"""Trainium2 Bass kernel for nn_CSATransformer_25778393710760.

Math: with this problem's parameters (all biases zero, ln affine identity),
the attention colsum scale cancels through LayerNorm (PFF is positively
homogeneous, colsum > 0), so
    out = LN(relu(x @ pfn_w1) @ pfn_w2 + x)
to ~1e-8.  Sharding: pure data parallel over batch B=8 across 8 cores.

Per-core kernel (L=4096 rows, D=128), fp16 matmul path (~5e-4 rel err,
gate is 2e-2), processed as 4 pairs of 1024 rows:
  SWDGE cast-DMA in (f32 HBM -> f16 SBUF, 4KB/partition lines; partition
  p holds rows 8p..8p+7) -> 8x PE transpose (f16 PSUM, LDW-bound) ->
  one DVE copy [128,1024] -> 2x w1 matmul -> one ACT relu [128,1024] ->
  2x (w2 matmul + residual as accumulating ident matmul) -> one ACT copy
  -> 8x PE transpose back (f16 PSUM) -> one DVE copy -> bn_stats +
  wide [128,8] stat combines -> sqrt/recip -> normalize split across
  ACT/DVE/GPSIMD -> HWDGE store (4KB lines).
The transpose column permutation induced by the DMA layout cancels
between the in- and out-transposes.  PE HAM warmup: 9 x N=512 matmuls
(~3.8us sustained) flip the clock gate to 2.4GHz before the pipeline.
"""

import os
import numpy as np

B, L, DX = 8, 4096, 128
_PAIRS = 4          # 1024-row blocks per core
_CPP = 8            # 128-row chunks per block
_WARM_MMS = 9       # N=512 PE warmup matmuls (~3.8us cold)
_BN3D = False       # grouped bn_stats crashes walrus (AP flattens); per chunk
# normalize chunk -> engine: 3x ACT, 2x DVE, 3x GPSIMD
_NORM_ENG = ("act", "act", "act", "dve", "dve", "gps", "gps", "gps")

_prog_cache = {}


def _build_program():
    import concourse.tile as tile
    from concourse import bacc, mybir
    from concourse.bass import ts

    f32 = mybir.dt.float32
    f16 = mybir.dt.float16
    AF = mybir.ActivationFunctionType
    OP = mybir.AluOpType

    nc = bacc.Bacc(None, target_bir_lowering=False)
    x = nc.dram_tensor("x", [L, DX], f32, kind="ExternalInput")
    w1 = nc.dram_tensor("w1", [DX, DX], f16, kind="ExternalInput")
    w2 = nc.dram_tensor("w2", [DX, DX], f16, kind="ExternalInput")
    identp = nc.dram_tensor("identp", [DX, DX], f16, kind="ExternalInput")
    y = nc.dram_tensor("y", [L, DX], f32, kind="ExternalOutput")

    with tile.TileContext(nc) as tc:
        with (
            tc.tile_pool(name="consts", bufs=1) as consts,
            tc.tile_pool(name="xg_pool", bufs=_PAIRS) as xg_pool,
            tc.tile_pool(name="work", bufs=4) as work,
            tc.tile_pool(name="pnp", bufs=_PAIRS) as pnp,
            tc.tile_pool(name="small", bufs=_PAIRS) as small,
            tc.tile_pool(name="io", bufs=3) as io,
            tc.tile_pool(name="ps_t", bufs=2, space="PSUM") as ps_t,
            tc.tile_pool(name="ps_mm", bufs=2, space="PSUM") as ps_mm,
            tc.tile_pool(name="ps_tb", bufs=2, space="PSUM") as ps_tb,
        ):
            # ---- tiny const DMAs first: everything gates on these ----
            ident_sb = consts.tile([128, 128], f16)
            w1_sb = consts.tile([128, 128], f16)
            w2_sb = consts.tile([128, 128], f16)
            nc.sync.dma_start(out=ident_sb, in_=identp[:, :])
            nc.scalar.dma_start(out=w1_sb, in_=w1[:, :])
            nc.scalar.dma_start(out=w2_sb, in_=w2[:, :])
            eps = consts.tile([128, 1], f32)
            nc.vector.memset(eps, 1e-6)
            warm_rhs = consts.tile([128, 512], f16)
            nc.vector.memset(warm_rhs, 0.5)

            # ---- issue all x cast-loads (f32 HBM -> f16 SBUF, SWDGE).
            # pair 0 in quarters so the first transposes start ASAP.
            xgs = []
            for g in range(_PAIRS):
                xg = xg_pool.tile([128, _CPP, 128], f16, tag="xg")
                src = x[ts(g, 1024), :].rearrange("(p r) d -> p r d", p=128)
                if g == 0:
                    for q in range(4):
                        nc.gpsimd.dma_start(
                            out=xg[:, ts(q, 2), :], in_=src[:, ts(q, 2), :]
                        )
                else:
                    nc.gpsimd.dma_start(out=xg, in_=src)
                xgs.append(xg)

            # ---- PE HAM warmup: ~3.8us of sustained matmul activity
            # flips the clock gate to 2.4GHz before the real pipeline.
            warm_ps = ps_mm.tile([128, 2, 512], f32, tag="mm")
            for _ in range(_WARM_MMS):
                nc.tensor.matmul(
                    warm_ps[:, 0, :], lhsT=ident_sb, rhs=warm_rhs,
                    start=True, stop=True,
                )
            warmsink = consts.tile([128, 1], f32)
            nc.vector.tensor_copy(out=warmsink, in_=warm_ps[:, 0, 0:1])
            # warm the ACT table set (sqrt anchor; relu/identity ride along)
            warm = consts.tile([128, 1], f32)
            nc.scalar.activation(out=warm, in_=eps, func=AF.Sqrt)
            nc.scalar.activation(out=warm, in_=eps, func=AF.Relu)
            nc.scalar.activation(out=warm, in_=eps, func=AF.Identity, bias=eps)

            # ---- stage-interleaved (wavefront) issue: each stage of
            # pair g is issued one wave after its producers, so every
            # strict per-engine FIFO always has ready work at its head
            # and later pairs' front halves never queue behind earlier
            # pairs' stats chains.
            st = [dict() for _ in range(_PAIRS)]

            def s0_tp_in(g):
                xtp = ps_t.tile([128, _CPP, 128], f16, tag="xtp")
                for c in range(_CPP):
                    nc.tensor.transpose(xtp[:, c, :], xgs[g][:, c, :], ident_sb)
                st[g]["xtp"] = xtp

            def s1_xt(g):
                xT = work.tile([128, 1024], f16, tag="xT")
                nc.vector.tensor_copy(
                    out=xT, in_=st[g]["xtp"].rearrange("p c d -> p (c d)")
                )
                st[g]["xT"] = xT

            def s2_mm1(g):
                y1p = ps_mm.tile([128, 2, 512], f32, tag="mm")
                for k in range(2):
                    nc.tensor.matmul(
                        y1p[:, k, :], lhsT=w1_sb, rhs=st[g]["xT"][:, ts(k, 512)],
                        start=True, stop=True,
                    )
                st[g]["y1p"] = y1p

            def s3_relu(g):
                y1s = work.tile([128, 1024], f16, tag="y1s")
                nc.scalar.activation(
                    out=y1s, in_=st[g]["y1p"].rearrange("p k n -> p (k n)"),
                    func=AF.Relu,
                )
                st[g]["y1s"] = y1s

            def s4_mm2(g):
                pp = ps_mm.tile([128, 2, 512], f32, tag="mm")
                for k in range(2):
                    nc.tensor.matmul(
                        pp[:, k, :], lhsT=w2_sb, rhs=st[g]["y1s"][:, ts(k, 512)],
                        start=True, stop=False,
                    )
                    nc.tensor.matmul(
                        pp[:, k, :], lhsT=ident_sb, rhs=st[g]["xT"][:, ts(k, 512)],
                        start=False, stop=True,
                    )
                st[g]["pp"] = pp

            def s5_y2(g):
                y2s = work.tile([128, 1024], f16, tag="y2s")
                nc.scalar.activation(
                    out=y2s, in_=st[g]["pp"].rearrange("p k n -> p (k n)"),
                    func=AF.Identity,
                )
                st[g]["y2s"] = y2s

            def s6_tp_back(g):
                ppT = ps_tb.tile([128, _CPP, 128], f16, tag="ppT")
                for c in range(_CPP):
                    nc.tensor.transpose(
                        ppT[:, c, :], st[g]["y2s"][:, ts(c, 128)], ident_sb
                    )
                st[g]["ppT"] = ppT

            def s7_pn(g):
                pn = pnp.tile([128, _CPP, 128], f16, tag="pn")
                nc.vector.tensor_copy(
                    out=pn.rearrange("p c d -> p (c d)"),
                    in_=st[g]["ppT"].rearrange("p c d -> p (c d)"),
                )
                st[g]["pn"] = pn

            def s8_bn(g):
                pn = st[g]["pn"]
                bstats = small.tile([128, _CPP, 6], f32, tag="bstats")
                if _BN3D:
                    nc.vector.bn_stats(out=bstats, in_=pn)
                else:
                    for c in range(_CPP):
                        nc.vector.bn_stats(out=bstats[:, c, :], in_=pn[:, c, :])
                # combine even/odd halves (counts equal 64):
                #   mean = (me+mo)/2;  var = (cve+cvo)/128 + ((me-mo)/2)^2
                me, mo = bstats[:, :, 1], bstats[:, :, 4]
                cve, cvo = bstats[:, :, 2], bstats[:, :, 5]
                sm = small.tile([128, _CPP], f32, tag="sm")
                nc.vector.tensor_add(out=sm, in0=me, in1=mo)
                dm = small.tile([128, _CPP], f32, tag="dm")
                nc.vector.tensor_sub(out=dm, in0=me, in1=mo)
                dsq = small.tile([128, _CPP], f32, tag="dsq")
                nc.vector.scalar_tensor_tensor(
                    out=dsq, in0=dm, scalar=0.25, in1=dm,
                    op0=OP.mult, op1=OP.mult,
                )
                vs = small.tile([128, _CPP], f32, tag="vs")
                nc.vector.tensor_add(out=vs, in0=cve, in1=cvo)
                var = small.tile([128, _CPP], f32, tag="var")
                nc.vector.scalar_tensor_tensor(
                    out=var, in0=vs, scalar=1.0 / 128.0, in1=dsq,
                    op0=OP.mult, op1=OP.add,
                )
                st[g]["sm"] = sm
                st[g]["var"] = var

            def s9_scales(g):
                std = small.tile([128, _CPP], f32, tag="std")
                nc.scalar.activation(
                    out=std, in_=st[g]["var"], func=AF.Sqrt, scale=1.0, bias=eps
                )
                rstd = small.tile([128, _CPP], f32, tag="rstd")
                nc.vector.reciprocal(out=rstd, in_=std)
                nmr = small.tile([128, _CPP], f32, tag="nmr")
                nc.vector.scalar_tensor_tensor(
                    out=nmr, in0=st[g]["sm"], scalar=-0.5, in1=rstd,
                    op0=OP.mult, op1=OP.mult,
                )
                st[g]["rstd"] = rstd
                st[g]["nmr"] = nmr

            def s10_norm_store(g):
                pn, rstd, nmr = st[g]["pn"], st[g]["rstd"], st[g]["nmr"]
                og = io.tile([128, _CPP, 128], f32, tag="og")
                for c in range(_CPP):
                    eng = _NORM_ENG[c]
                    if eng == "act":
                        nc.scalar.activation(
                            out=og[:, c, :], in_=pn[:, c, :], func=AF.Identity,
                            bias=nmr[:, c : c + 1], scale=rstd[:, c : c + 1],
                        )
                    else:
                        veng = nc.vector if eng == "dve" else nc.gpsimd
                        veng.tensor_scalar(
                            out=og[:, c, :], in0=pn[:, c, :],
                            scalar1=rstd[:, c : c + 1], scalar2=nmr[:, c : c + 1],
                            op0=OP.mult, op1=OP.add,
                        )
                dst = y[ts(g, 1024), :].rearrange("(p r) d -> p r d", p=128)
                deng = nc.sync if g % 2 == 0 else nc.scalar
                deng.dma_start(out=dst, in_=og)

            stages = [
                s0_tp_in, s1_xt, s2_mm1, s3_relu, s4_mm2, s5_y2,
                s6_tp_back, s7_pn, s8_bn, s9_scales, s10_norm_store,
            ]
            # within a wave, issue oldest pairs' late stages first so each
            # engine FIFO head is always a dependency-satisfied op
            for wave in range(_PAIRS + len(stages) - 1):
                for s in reversed(range(len(stages))):
                    g = wave - s
                    if 0 <= g < _PAIRS:
                        stages[s](g)
    nc.finalize()
    return nc


def _ensure_ntff_hook():
    """Register the axon NTFF profiling hook if the image lacks antenv.axon_hooks."""
    try:
        from antenv.axon_hooks import get_axon_ntff_profile_hook  # noqa: F401
        return
    except ImportError:
        pass
    import sys
    import types

    import antenv
    from trn_agent_boot.trn_boot import _ntff_profile_via_ctypes

    hook = _ntff_profile_via_ctypes("/opt/axon/libaxon_pjrt.so")
    mod = types.ModuleType("antenv.axon_hooks")
    mod._hook = hook
    mod.set_axon_ntff_profile_hook = lambda h: setattr(mod, "_hook", h)
    mod.get_axon_ntff_profile_hook = lambda: mod._hook
    sys.modules["antenv.axon_hooks"] = mod
    antenv.axon_hooks = mod


def _run_device(x, w1, w2, trace=False):
    import concourse.bass_utils as bass_utils
    from concourse.bass_utils import run_bass_kernel_spmd

    if trace:
        try:
            _ensure_ntff_hook()
            bass_utils.upload_artifacts = lambda tmpdir: str(tmpdir)
        except Exception as e:  # profiling is best-effort
            print(f"ntff hook unavailable ({e}); running without trace")
            trace = False

    if "prog" not in _prog_cache:
        _prog_cache["prog"] = _build_program()
    nc = _prog_cache["prog"]
    w1h = np.ascontiguousarray(w1, dtype=np.float16)
    w2h = np.ascontiguousarray(w2, dtype=np.float16)
    identh = np.eye(DX, dtype=np.float16)
    in_maps = [
        {
            "x": np.ascontiguousarray(x[b], dtype=np.float32),
            "w1": w1h,
            "w2": w2h,
            "identp": identh,
        }
        for b in range(B)
    ]
    res = run_bass_kernel_spmd(
        nc, in_maps, core_ids=list(range(B)), trace=trace,
        trace_cores=list(range(B)) if trace else None,
    )
    kernel.last_result = res
    kernel.last_exec_time_ns = res.exec_time_ns
    return np.stack([r["y"] for r in res.results], axis=0)


def _numpy_fallback(inputs):
    """Faithful (but slow) mirror of the reference for unexpected inputs."""
    f32 = np.float32
    x = np.asarray(inputs["x"], f32)
    c = np.asarray(inputs["c"], f32)
    W1 = np.asarray(inputs["W1"], f32); W2 = np.asarray(inputs["W2"], f32)
    wt_w = np.asarray(inputs["wt_w"], f32); bsa = np.asarray(inputs["bsa"], f32)
    Wsa1 = np.asarray(inputs["Wsa1"], f32); Wsa2 = np.asarray(inputs["Wsa2"], f32)
    wsat_w = np.asarray(inputs["wsat_w"], f32)
    wsat_b = np.asarray(inputs["wsat_b"], f32); bsa1 = np.asarray(inputs["bsa1"], f32)
    pfn_w1 = np.asarray(inputs["pfn_w1"], f32); pfn_b1 = np.asarray(inputs["pfn_b1"], f32)
    pfn_w2 = np.asarray(inputs["pfn_w2"], f32); pfn_b2 = np.asarray(inputs["pfn_b2"], f32)
    ln_g = np.asarray(inputs["ln_g"], f32); ln_b = np.asarray(inputs["ln_b"], f32)
    Bs, Ls, _ = x.shape
    wx = x @ W1
    wq = c @ W2
    logits = (wx + wq[:, None, :] + bsa) @ wt_w
    m = logits.max(-1, keepdims=True)
    e = np.exp(logits - m)
    p = (e / e.sum(-1, keepdims=True))[..., None]
    h = x * p
    si = (h @ Wsa1) @ wsat_w
    sj = (h @ Wsa2) @ wsat_w
    const = bsa1 @ wsat_w + wsat_b
    colsum = np.zeros((Bs, Ls), f32)
    blk = 512
    for b in range(Bs):
        for i0 in range(0, Ls, blk):
            s = 1.0 / (1.0 + np.exp(-(si[b, i0 : i0 + blk, None] + sj[b, None, :] + const)))
            for r in range(s.shape[0]):
                s[r, i0 + r] = -np.inf
            sm = s.max(-1, keepdims=True)
            ee = np.exp(s - sm)
            colsum[b] += (ee / ee.sum(-1, keepdims=True)).sum(0)
    ui = x * colsum[..., None]
    yv = np.maximum(ui @ pfn_w1 + pfn_b1, 0.0)
    yv = yv @ pfn_w2 + pfn_b2 + ui
    mu = yv.mean(-1, keepdims=True)
    var = ((yv - mu) ** 2).mean(-1, keepdims=True)
    return ((yv - mu) / np.sqrt(var + 1e-6) * ln_g + ln_b).astype(f32)


def kernel(**inputs):
    x = np.asarray(inputs["x"], dtype=np.float32)
    pfn_w1 = np.asarray(inputs["pfn_w1"], dtype=np.float32)
    pfn_w2 = np.asarray(inputs["pfn_w2"], dtype=np.float32)

    fast_ok = (
        x.shape == (B, L, DX)
        and not np.any(np.asarray(inputs["pfn_b1"]))
        and not np.any(np.asarray(inputs["pfn_b2"]))
        and np.all(np.asarray(inputs["ln_g"]) == 1.0)
        and not np.any(np.asarray(inputs["ln_b"]))
    )
    if not fast_ok:
        return _numpy_fallback(inputs)

    trace = bool(int(os.environ.get("CSA_TRACE", "0")))
    return _run_device(x, pfn_w1, pfn_w2, trace=trace)


kernel.last_exec_time_ns = None
kernel.last_result = None
